# revision 8
# baseline (speedup 1.0000x reference)
"""Sparse multi-head self-attention (sliding window + global columns) on 8
Trainium2 NeuronCores.

Sharding: fully data-parallel over the sequence dimension. Core c produces
output rows [512c, 512c+512). Each core recomputes k/v for a 128-row halo on
each side of its slice plus the 16 global key rows (j % 256 == 0), so no
collectives are needed. All matmuls run as float32r (full-rate fp32 on the PE,
~2e-4 relative rounding per matmul).

RoPE trick: the rotary pairing (even/odd interleave) is turned into contiguous
halves by permuting the D axis of x and the input columns of in_proj_w
identically on the host; the rotation then uses contiguous 32-wide slices.
"""
import sys

sys.path.insert(0, "/opt/trn_rl_repo")

import numpy as np
import concourse.bass as bass
import concourse.mybir as mybir
from concourse.tile import TileContext
from concourse.vector_clock import ScopedClock  # noqa: F401 (import check)

# ---------------------------------------------------------------- constants
B, T, D = 1, 4096, 1024
H, HD = 16, 64
W = 128
GSTRIDE = 256
ROPE_BASE = 10000.0
NCORES = 8
TLOC = T // NCORES            # 512 own rows per core
HALO = 128
NL = TLOC + 2 * HALO          # 768 rows incl. halo
NG = T // GSTRIDE             # 16 global keys
NT = NL // 128                # 6 local 128-row tiles
NQB = TLOC // 128             # 4 query blocks per core
NDT = D // 128                # 8 din tiles
SCALE = 1.0 / np.sqrt(HD)

# per-m query-column ranges within the core's 512 own rows
QS = [0, 0, 0, 128, 256, 384]
QW = [128, 256, 384, 384, 256, 128]

DT = mybir.dt.float32
F32R = mybir.dt.float32r
FT = mybir.ActivationFunctionType

_cache = {}


# ------------------------------------------------------- walrus workaround
def _fix_multi_waits(nc):
    """This walrus build encodes at most ONE sem wait per instruction; hoist
    extra waits onto same-engine NoOps inserted just before the owner."""
    count = 0
    for fn in nc.m.functions:
        for bb in fn.blocks:
            old = bb.instructions
            if not any(
                i.sync_info is not None and len(i.sync_info.on_wait or []) > 1
                for i in old
            ):
                continue
            new = []
            for inst in old:
                si = inst.sync_info
                waits = list(si.on_wait) if si is not None and si.on_wait else []
                if len(waits) > 1:
                    for w in waits[:-1]:
                        count += 1
                        new.append(
                            mybir.InstNoOp(
                                name=f"I-waitfix-{count}",
                                engine=inst.engine,
                                bass_nofuse=True,
                                sync_info=mybir.SyncInfo(on_wait=[w], on_update=[]),
                            )
                        )
                    inst.sync_info = mybir.SyncInfo(
                        on_wait=[waits[-1]], on_update=list(si.on_update or [])
                    )
                new.append(inst)
            bb.instructions = new
    return count


def _bcast_mid(ap2d, reps):
    """[P, F] AP -> [P, reps, F] AP broadcasting along a middle free dim."""
    a = [list(x) for x in ap2d.ap]
    return bass.AP(tensor=ap2d.tensor, offset=ap2d.offset,
                   ap=[a[0], [0, reps], a[1]])


# ------------------------------------------------------------ bass program
def build_program():
    nc = bass.Bass()

    xl = nc.dram_tensor("xl", [NL, D], DT, kind="ExternalInput")
    xg = nc.dram_tensor("xg", [NG, D], DT, kind="ExternalInput")
    wt = nc.dram_tensor("wt", [D, 3 * D], F32R, kind="ExternalInput")
    wo = nc.dram_tensor("wo", [D, D], F32R, kind="ExternalInput")
    bqk = nc.dram_tensor("bqk", [128, 16], DT, kind="ExternalInput")   # q|k chunk biases
    bv = nc.dram_tensor("bv", [1, D], F32R, kind="ExternalInput")
    bo = nc.dram_tensor("bo", [1, D], F32R, kind="ExternalInput")
    csl = nc.dram_tensor("csl", [NL, 64], DT, kind="ExternalInput")    # cos|sin
    csg = nc.dram_tensor("csg", [NG, 64], DT, kind="ExternalInput")
    mloc = nc.dram_tensor("mloc", [NT, 128, 384], DT, kind="ExternalInput")
    mglob = nc.dram_tensor("mglob", [NG, TLOC], DT, kind="ExternalInput")
    identd = nc.dram_tensor("identd", [128, 128], F32R, kind="ExternalInput")
    out = nc.dram_tensor("out", [TLOC, D], DT, kind="ExternalOutput")

    with TileContext(nc) as tc:
        _build_body(nc, tc, xl, xg, wt, wo, bqk, bv, bo, csl, csg,
                    mloc, mglob, identd, out)
    _fix_multi_waits(nc)
    return nc


def _build_body(nc, tc, xl, xg, wt, wo, bqk, bv, bo, csl, csg,
                mloc, mglob, identd, out):
    from contextlib import ExitStack
    ctx = ExitStack()
    with ctx:
        singles = ctx.enter_context(tc.tile_pool(name="singles", bufs=1))
        xpool = ctx.enter_context(tc.tile_pool(name="xpool", bufs=2))
        rpool = ctx.enter_context(tc.tile_pool(name="rpool", bufs=2))
        tpool = ctx.enter_context(tc.tile_pool(name="tpool", bufs=4))
        wpool = ctx.enter_context(tc.tile_pool(name="wpool", bufs=3))
        ppool = ctx.enter_context(tc.tile_pool(name="ppool", bufs=3))
        npool = ctx.enter_context(tc.tile_pool(name="npool", bufs=2))
        opool = ctx.enter_context(tc.tile_pool(name="opool", bufs=2))
        ps_mm = ctx.enter_context(tc.tile_pool(name="ps_mm", bufs=3, space="PSUM"))
        ps_s = ctx.enter_context(tc.tile_pool(name="ps_s", bufs=3, space="PSUM"))
        ps_o = ctx.enter_context(tc.tile_pool(name="ps_o", bufs=2, space="PSUM"))

        # ---------------- constants
        ident = singles.tile([128, 128], F32R)
        nc.sync.dma_start(ident[:], identd[:])
        ones = singles.tile([1, 128], F32R)
        nc.vector.memset(ones[:].bitcast(DT), 1.0)
        bqk_sb = singles.tile([128, 16], DT)
        nc.sync.dma_start(bqk_sb[:], bqk[:])
        bv_sb = singles.tile([1, D], F32R)
        nc.sync.dma_start(bv_sb[:], bv[:])
        bo_sb = singles.tile([1, D], F32R)
        nc.sync.dma_start(bo_sb[:], bo[:])
        mask_sb = []
        for m in range(NT):
            t = singles.tile([128, 384], DT, tag=f"mask{m}", name=f"mask{m}")
            nc.sync.dma_start(t[:], mloc[m])
            mask_sb.append(t)
        mg_sb = singles.tile([NG, TLOC], DT)
        nc.sync.dma_start(mg_sb[:], mglob[:])

        # ---------------- RoPE (local rows) + transpose to xT
        xT = [singles.tile([128, NL], F32R, tag=f"xT{k}", name=f"xT{k}")
              for k in range(NDT)]

        def rope(x_sb, cs_sb, roped, tmp, p):
            """x_sb [p, D] (perm layout), cs_sb [p, 64] -> roped [p, D] f32r."""
            x3 = x_sb[:p].rearrange("p (h d) -> p h d", h=H)
            r3 = roped[:p].rearrange("p (h d) -> p h d", h=H)
            t3 = tmp[:p].rearrange("p (h d) -> p h d", h=H)
            cosb = _bcast_mid(cs_sb[:p, 0:32], H)
            sinb = _bcast_mid(cs_sb[:p, 32:64], H)
            xe = x3[:, :, 0:32]
            xo = x3[:, :, 32:64]
            # even half: xe*cos - xo*sin ; odd half: xe*sin + xo*cos
            nc.vector.tensor_mul(t3[:, :, 0:32], xe, cosb)
            nc.vector.tensor_mul(t3[:, :, 32:64], xo, sinb)
            nc.vector.tensor_sub(r3[:, :, 0:32], t3[:, :, 0:32], t3[:, :, 32:64])
            nc.vector.tensor_mul(t3[:, :, 0:32], xe, sinb)
            nc.vector.tensor_mul(t3[:, :, 32:64], xo, cosb)
            nc.vector.tensor_add(r3[:, :, 32:64], t3[:, :, 0:32], t3[:, :, 32:64])

        for i in range(NT):
            x_sb = xpool.tile([128, D], DT, tag="x")
            nc.sync.dma_start(x_sb[:], xl[i * 128:(i + 1) * 128, :])
            cs_sb = xpool.tile([128, 64], DT, tag="cs")
            nc.sync.dma_start(cs_sb[:], csl[i * 128:(i + 1) * 128, :])
            roped = rpool.tile([128, D], F32R, tag="roped")
            tmp = rpool.tile([128, D], DT, tag="ropetmp")
            rope(x_sb, cs_sb, roped, tmp, 128)
            for k in range(NDT):
                ptr = ps_mm.tile([128, 128], F32R, tag="mm")
                nc.tensor.transpose(ptr[:], roped[:, k * 128:(k + 1) * 128],
                                    ident[:])
                nc.vector.tensor_copy(xT[k][:, i * 128:(i + 1) * 128], ptr[:])

        # ---------------- RoPE + transpose for global rows (reuse pools)
        xTg = singles.tile([128, NDT, NG], F32R)
        xg_sb = xpool.tile([NG, D], DT, tag="x")
        nc.sync.dma_start(xg_sb[:], xg[:])
        csg_sb = xpool.tile([NG, 64], DT, tag="cs")
        nc.sync.dma_start(csg_sb[:], csg[:])
        ropedg = rpool.tile([NG, D], F32R, tag="roped")
        tmpg = rpool.tile([NG, D], DT, tag="ropetmp")
        rope(xg_sb, csg_sb, ropedg, tmpg, NG)
        for k in range(NDT):
            ptr = ps_mm.tile([128, NG], F32R, tag="mm")
            nc.tensor.transpose(ptr[:], ropedg[:, k * 128:(k + 1) * 128],
                                ident[0:NG, 0:NG])
            nc.vector.tensor_copy(xTg[:, k, :], ptr[:])

        # ---------------- qkv projections (qT/kT orientation; v natural)
        qT = [singles.tile([128, TLOC], F32R, tag=f"qT{ch}", name=f"qT{ch}")
              for ch in range(NDT)]
        kT = [singles.tile([128, NL + NG], F32R, tag=f"kT{ch}", name=f"kT{ch}")
              for ch in range(NDT)]

        for ch in range(NDT):
            # q chunk: dout rows [128ch, 128ch+128) of wt cols 0:1024
            pq = ps_mm.tile([128, 512], DT, tag="mm")
            for k in range(NDT):
                wtile = wpool.tile([128, 128], F32R, tag="wqk")
                nc.sync.dma_start(
                    wtile[:], wt[k * 128:(k + 1) * 128, ch * 128:ch * 128 + 128])
                nc.tensor.matmul(pq[:], wtile[:], xT[k][:, HALO:HALO + TLOC],
                                 start=(k == 0), stop=(k == NDT - 1))
            nc.scalar.add(qT[ch][:], pq[:], bqk_sb[:, ch:ch + 1])

            # k chunk: dout rows 1024 + [128ch, 128ch+128), two 384-wide t-chunks
            for tch in range(2):
                pk = ps_mm.tile([128, 512], DT, tag="mm")
                for k in range(NDT):
                    wtile = wpool.tile([128, 128], F32R, tag="wqk")
                    nc.sync.dma_start(
                        wtile[:],
                        wt[k * 128:(k + 1) * 128,
                           D + ch * 128:D + ch * 128 + 128])
                    nc.tensor.matmul(
                        pk[:, 0:384], wtile[:],
                        xT[k][:, tch * 384:(tch + 1) * 384],
                        start=(k == 0), stop=(k == NDT - 1))
                nc.scalar.add(kT[ch][:, tch * 384:(tch + 1) * 384],
                              pk[:, 0:384], bqk_sb[:, 8 + ch:9 + ch])
            # k globals -> kT cols NL:NL+NG
            pkg = ps_mm.tile([128, 512], DT, tag="mm")
            for k in range(NDT):
                wtile = wpool.tile([128, 128], F32R, tag="wqk")
                nc.sync.dma_start(
                    wtile[:],
                    wt[k * 128:(k + 1) * 128, D + ch * 128:D + ch * 128 + 128])
                nc.tensor.matmul(pkg[:, 0:NG], wtile[:], xTg[:, k, :],
                                 start=(k == 0), stop=(k == NDT - 1))
            nc.scalar.add(kT[ch][:, NL:NL + NG], pkg[:, 0:NG],
                          bqk_sb[:, 8 + ch:9 + ch])

        # v natural [t, 16 heads, 64+1]; ones col at the end of each head slot
        v_sb = [singles.tile([128, H, HD + 1], F32R, tag=f"v{m}", name=f"v{m}")
                for m in range(NT)]
        for m in range(NT):
            nc.vector.memset(v_sb[m][:, :, HD:HD + 1].bitcast(DT), 1.0)
            for ch in range(2):
                pv = ps_mm.tile([128, 512], DT, tag="mm")
                for k in range(NDT):
                    wtile = wpool.tile([128, 512], F32R, tag="wv")
                    nc.sync.dma_start(
                        wtile[:],
                        wt[k * 128:(k + 1) * 128,
                           2 * D + ch * 512:2 * D + ch * 512 + 512])
                    nc.tensor.matmul(pv[:], xT[k][:, m * 128:(m + 1) * 128],
                                     wtile[:], start=(k == 0), stop=False)
                nc.tensor.matmul(pv[:], ones[:, 0:128],
                                 bv_sb[:, ch * 512:(ch + 1) * 512],
                                 start=False, stop=True)
                nc.vector.tensor_copy(
                    v_sb[m][:, ch * 8:(ch + 1) * 8, 0:HD],
                    pv[:].rearrange("p (h d) -> p h d", h=8))

        vg_sb = singles.tile([NG, H, HD + 1], F32R)
        nc.vector.memset(vg_sb[:, :, HD:HD + 1].bitcast(DT), 1.0)
        for ch in range(2):
            pv = ps_mm.tile([128, 512], DT, tag="mm")
            for k in range(NDT):
                wtile = wpool.tile([128, 512], F32R, tag="wv")
                nc.sync.dma_start(
                    wtile[:],
                    wt[k * 128:(k + 1) * 128,
                       2 * D + ch * 512:2 * D + ch * 512 + 512])
                nc.tensor.matmul(pv[0:NG, :], xTg[:, k, :], wtile[:],
                                 start=(k == 0), stop=False)
            nc.tensor.matmul(pv[0:NG, :], ones[:, 0:NG],
                             bv_sb[:, ch * 512:(ch + 1) * 512],
                             start=False, stop=True)
            nc.vector.tensor_copy(
                vg_sb[:, ch * 8:(ch + 1) * 8, 0:HD],
                pv[0:NG, :].rearrange("p (h d) -> p h d", h=8))

        # ---------------- attention per head
        oT = [singles.tile([128, TLOC], F32R, tag=f"oT{k}", name=f"oT{k}")
              for k in range(NDT)]
        ostage = singles.tile([64, TLOC], F32R)

        for h in range(H):
            hp, off = h // 2, 64 * (h % 2)
            qh = qT[hp][off:off + 64, :]
            kh = kT[hp][off:off + 64, :]
            po = ps_o.tile([65, TLOC], DT, tag="o")
            for m in range(NT):
                w, qs = QW[m], QS[m]
                psc = ps_s.tile([128, 512], DT, tag="s")
                nc.tensor.matmul(psc[:, 0:w], kh[:, m * 128:(m + 1) * 128],
                                 qh[:, qs:qs + w], start=True, stop=True)
                nc.vector.tensor_add(psc[:, 0:w], psc[:, 0:w],
                                     mask_sb[m][:, 0:w])
                pe = ppool.tile([128, 384], F32R, tag="pexp")
                nc.scalar.activation(pe[:, 0:w], psc[:, 0:w], FT.Exp)
                nc.tensor.matmul(po[:, qs:qs + w], v_sb[m][:, h, :],
                                 pe[:, 0:w], start=(m == 0), stop=False)
            # globals
            psg = ps_s.tile([NG, 512], DT, tag="s")
            nc.tensor.matmul(psg[:, 0:TLOC], kh[:, NL:NL + NG], qh[:],
                             start=True, stop=True)
            nc.vector.tensor_add(psg[:, 0:TLOC], psg[:, 0:TLOC], mg_sb[:])
            peg = ppool.tile([NG, 512], F32R, tag="pexpg")
            nc.scalar.activation(peg[:, 0:TLOC], psg[:, 0:TLOC], FT.Exp)
            nc.tensor.matmul(po[:], vg_sb[:, h, :], peg[:, 0:TLOC],
                             start=False, stop=True)

            # softmax denominators: row 64 of po; newton-refined reciprocal
            rcp0 = npool.tile([1, TLOC], DT, tag="rcp0")
            t1 = npool.tile([1, TLOC], DT, tag="rtmp")
            rcp = npool.tile([1, TLOC], F32R, tag="rcp")
            nc.vector.reciprocal(rcp0[:], po[64:65, :])
            nc.vector.tensor_mul(t1[:], po[64:65, :], rcp0[:])
            nc.vector.tensor_scalar(t1[:], t1[:], -1.0, 2.0,
                                    mybir.AluOpType.mult, mybir.AluOpType.add)
            nc.vector.tensor_mul(rcp[:], rcp0[:], t1[:])
            pb = ps_s.tile([64, 512], DT, tag="s")
            nc.tensor.matmul(pb[:, 0:TLOC], ones[:, 0:64], rcp[:],
                             start=True, stop=True)
            bc = npool.tile([64, TLOC], DT, tag="bc")
            nc.vector.tensor_copy(bc[:], pb[:, 0:TLOC])
            if off == 0:
                nc.vector.tensor_mul(oT[hp][0:64, :], po[0:64, :], bc[:])
            else:
                nc.vector.tensor_mul(ostage[:], po[0:64, :], bc[:])
                nc.sync.dma_start(oT[hp][64:128, :], ostage[:])

        # ---------------- output projection
        for qb in range(NQB):
            for ch in range(2):
                pout = ps_mm.tile([128, 512], DT, tag="mm")
                for k in range(NDT):
                    wtile = wpool.tile([128, 512], F32R, tag="wo")
                    nc.sync.dma_start(
                        wtile[:],
                        wo[k * 128:(k + 1) * 128, ch * 512:(ch + 1) * 512])
                    nc.tensor.matmul(pout[:], oT[k][:, qb * 128:(qb + 1) * 128],
                                     wtile[:], start=(k == 0), stop=False)
                nc.tensor.matmul(pout[:], ones[:, 0:128],
                                 bo_sb[:, ch * 512:(ch + 1) * 512],
                                 start=False, stop=True)
                so = opool.tile([128, 512], DT, tag="outsb")
                nc.vector.tensor_copy(so[:], pout[:])
                nc.sync.dma_start(
                    out[qb * 128:(qb + 1) * 128, ch * 512:(ch + 1) * 512],
                    so[:])


# ------------------------------------------------------------ host helpers
def _perm():
    p = np.arange(D).reshape(H, 32, 2)
    return np.concatenate([p[:, :, 0], p[:, :, 1]], axis=1).reshape(-1)


def _cos_sin(trows):
    """Tables matching the reference's quirky emb[..., ::2] indexing."""
    inv_freq = (1.0 / (ROPE_BASE ** (np.arange(0, HD, 2, dtype=np.float32) / HD))
                ).astype(np.float32)
    pos = trows.astype(np.float32)
    freqs = pos[:, None] * inv_freq[None, :]
    emb = np.concatenate([freqs, freqs], axis=-1)[:, ::2]      # (n, 32)
    return np.cos(emb).astype(np.float32), np.sin(emb).astype(np.float32)


def _allowed(i, j):
    ok = (np.abs(i - j) <= W) | (j % GSTRIDE == 0) | (j == 0)
    return ok & (j >= 0) & (j < T)


def make_in_maps(x, in_proj_w, in_proj_b, out_w, out_b):
    perm = _perm()
    x2 = np.ascontiguousarray(np.asarray(x, np.float32).reshape(T, D)[:, perm])
    wp = np.asarray(in_proj_w, np.float32)[:, perm]
    wt_full = np.ascontiguousarray(wp.T).astype(np.float32)     # (D, 3D)
    wt_full[:, 0:D] *= SCALE
    b = np.asarray(in_proj_b, np.float32).copy()
    bq = (b[0:D] * SCALE).reshape(NDT, 128).T                   # (128, 8)
    bk = b[D:2 * D].reshape(NDT, 128).T
    bqk = np.ascontiguousarray(np.concatenate([bq, bk], axis=1))  # (128,16)
    bv = np.ascontiguousarray(b[2 * D:3 * D][None, :])
    wo_full = np.ascontiguousarray(np.asarray(out_w, np.float32).T)
    bo = np.ascontiguousarray(np.asarray(out_b, np.float32)[None, :])
    ident = np.eye(128, dtype=np.float32)

    tg = np.arange(NG) * GSTRIDE
    xg = np.ascontiguousarray(x2[tg])
    cg, sg = _cos_sin(tg)
    csg = np.ascontiguousarray(np.concatenate([cg, sg], axis=1))

    in_maps = []
    for c in range(NCORES):
        t0 = c * TLOC - HALO
        rows = np.arange(t0, t0 + NL)
        valid = (rows >= 0) & (rows < T)
        xlc = np.zeros((NL, D), np.float32)
        xlc[valid] = x2[rows[valid]]
        cl, sl = _cos_sin(np.clip(rows, 0, T - 1))
        csl = np.ascontiguousarray(np.concatenate([cl, sl], axis=1))

        # local masks: m-th key block vs the valid 384-col query window
        ml = np.full((NT, 128, 384), -1e30, np.float32)
        for m in range(NT):
            jj = (t0 + m * 128) + np.arange(128)
            ii = c * TLOC + QS[m] + np.arange(QW[m])
            ml[m, :, 0:QW[m]] = np.where(
                _allowed(ii[None, :], jj[:, None]), 0.0, -1e30)
        # global mask: kill global keys already covered by the local window
        iq = c * TLOC + np.arange(TLOC)
        qb = iq // 128
        jg = tg[:, None]
        covered = (jg >= 128 * (qb[None, :] - 1)) & (jg < 128 * (qb[None, :] + 2))
        mgl = np.where(covered, -1e30, 0.0).astype(np.float32)

        in_maps.append({
            "xl": xlc, "xg": xg, "wt": wt_full, "wo": wo_full,
            "bqk": bqk, "bv": bv, "bo": bo, "csl": csl, "csg": csg,
            "mloc": ml, "mglob": mgl, "identd": ident,
        })
    return in_maps


def kernel(x, in_proj_w, in_proj_b, out_w, out_b):
    from concourse.bass_utils import run_bass_kernel_spmd

    if "nc" not in _cache:
        _cache["nc"] = build_program()
    nc = _cache["nc"]
    in_maps = make_in_maps(x, in_proj_w, in_proj_b, out_w, out_b)
    res = run_bass_kernel_spmd(nc, in_maps, list(range(NCORES))).results
    pieces = [res[c]["out"] for c in range(NCORES)]
    return np.concatenate(pieces, axis=0).reshape(B, T, D).astype(np.float32)
